# revision 31
# baseline (speedup 1.0000x reference)
"""Multi-head attention (B=2, S=2048, D=1024, H=16) on 8 trn2 NeuronCores.

Sharding: data-parallel over batch (2) x tensor-parallel over head-groups (4).
Each core handles one batch and 4 heads (256 model dims).

Key optimizations:
  - Key compaction: masked-out keys contribute exactly 0 to attention, so the
    host gathers only unmasked key/value positions (padded to a multiple of
    128, with -1e9 exp-bias on pad slots). Halves K/V projection, scores,
    exp, and AV work for a ~50% random mask.
  - Transposed attention: QT/KT projections with head-dim on partitions,
    scores computed as S^T = K_h @ Q_h^T per 128-key chunk, so the AV matmul
    consumes exp(S^T) directly with no transposes anywhere.
  - Head pairs: the two heads of an e-chunk occupy partitions 0-63/64-127;
    their K=64 score matmuls go to PE row-groups (0,0)/(64,0) and run
    concurrently; one ScalarE exp instruction covers both heads (N=1024).
  - Mask folded into exp: per-partition bias (0/-1e9) on the activation.
  - AV with M=65: V augmented with a ones column computes the softmax
    denominator in the same matmul; normalize via DVE reciprocal + a small
    DRAM-roundtrip partition-broadcast (DMA with partition-stride 0).
  - All matmuls bf16 (host pre-casts/pre-transposes), fp32 PSUM; the 1/8
    attention scale is folded into Wq on the host.

Host sums the 4 partial y's per batch and adds bo.
"""

import numpy as np
import ml_dtypes

B, S, D, H, DH = 2, 2048, 1024, 16, 64
G = 4              # head-groups == cores per batch
EG = D // G        # 256 dims per group
NHG = H // G       # 4 heads per group
P = 128
NDC = D // P       # 8 contraction chunks for projections
NEC = EG // P      # 2 e-chunks (head pairs)
NQT = 4            # q-tiles of 512
QT = S // NQT      # 512

BF16 = ml_dtypes.bfloat16

_CACHE = {}


def _build(sk, reps=1):
    key = (sk, reps)
    if key in _CACHE:
        return _CACHE[key]
    import concourse.bacc as bacc
    import concourse.bass as bass
    import concourse.mybir as mybir
    import concourse.tile as tile

    assert sk % P == 0
    nkc = sk // P          # key chunks
    dt = mybir.dt
    F32 = dt.float32
    BF = dt.bfloat16
    Alu = mybir.AluOpType
    Act = mybir.ActivationFunctionType

    nc = bacc.Bacc("TRN2", target_bir_lowering=False, debug=False)

    xqT = nc.dram_tensor("xqT", [D, S], BF, kind="ExternalInput")
    xkT = nc.dram_tensor("xkT", [D, sk], BF, kind="ExternalInput")
    xvT = nc.dram_tensor("xvT", [D, sk], BF, kind="ExternalInput")
    wqT = nc.dram_tensor("wqT", [D, EG], BF, kind="ExternalInput")
    wkT = nc.dram_tensor("wkT", [D, EG], BF, kind="ExternalInput")
    wvT = nc.dram_tensor("wvT", [D, EG], BF, kind="ExternalInput")
    woT = nc.dram_tensor("woT", [EG, D], BF, kind="ExternalInput")
    bqv = nc.dram_tensor("bqv", [EG], F32, kind="ExternalInput")
    bkv = nc.dram_tensor("bkv", [EG], F32, kind="ExternalInput")
    bvv = nc.dram_tensor("bvv", [EG], F32, kind="ExternalInput")
    mbv = nc.dram_tensor("mbv", [sk], F32, kind="ExternalInput")
    y = nc.dram_tensor("y", [S, D], BF, kind="ExternalOutput")

    # s-tile widths for the key-side projections (pad tail to 128-multiples)
    kst = []
    off = 0
    while off < sk:
        w = min(512, sk - off)
        kst.append((off, w))
        off += w

    big = nkc <= 12   # leaner pools for large key counts (SBUF budget)

    with tile.TileContext(nc) as tc:
        with (
            tc.tile_pool(name="consts", bufs=1) as consts,
            tc.tile_pool(name="inp", bufs=20 if big else 12) as inp,
            tc.tile_pool(name="ptp", bufs=2 * nkc + 6 if big else nkc + 4) as ptp,
            tc.tile_pool(name="small", bufs=6 if big else 3) as small,
            tc.tile_pool(name="psmm", bufs=2, space="PSUM") as psmm,
            tc.tile_pool(name="pssc", bufs=2, space="PSUM") as pssc,
            tc.tile_pool(name="pso", bufs=2, space="PSUM") as psop,
            tc.tile_pool(name="dscr", bufs=2, space="DRAM") as dscr,
        ):
            # ---- constants / weights ----
            wk_sb = consts.tile([P, NDC, EG], BF)
            nc.sync.dma_start(out=wk_sb, in_=wkT[:].rearrange("(c p) e -> p c e", p=P))
            wv_sb = consts.tile([P, NDC, EG], BF)
            nc.sync.dma_start(out=wv_sb, in_=wvT[:].rearrange("(c p) e -> p c e", p=P))
            bq_sb = consts.tile([P, NEC], F32)
            nc.sync.dma_start(out=bq_sb, in_=bqv[:].rearrange("(c p) -> p c", p=P))
            bk_sb = consts.tile([P, NEC], F32)
            nc.sync.dma_start(out=bk_sb, in_=bkv[:].rearrange("(c p) -> p c", p=P))
            bv_sb = consts.tile([P, EG], F32)
            nc.sync.dma_start(
                out=bv_sb, in_=bass.AP(tensor=bvv, offset=0, ap=[[0, P], [1, EG]])
            )
            mb_sb = consts.tile([P, nkc], F32)
            nc.sync.dma_start(out=mb_sb, in_=mbv[:].rearrange("(c p) -> p c", p=P))

            wq_sb = consts.tile([P, NDC, EG], BF)
            nc.sync.dma_start(out=wq_sb, in_=wqT[:].rearrange("(c p) e -> p c e", p=P))
            wo_sb = consts.tile([P, NEC, D], BF)
            nc.sync.dma_start(out=wo_sb, in_=woT[:].rearrange("(c p) e -> p c e", p=P))
            # persistent activations
            qt_sb = consts.tile([P, NEC, S], BF)      # Q^T (x 1/8 via host Wq)
            kt_sb = consts.tile([P, NEC, sk], BF)     # K^T over compacted keys
            va_sb = consts.tile([P, nkc, NHG, DH + 1], BF)  # V + ones column
            ot_sb = consts.tile([P, NEC, S], BF)      # attention out^T

            nc.vector.memset(va_sb[:, :, :, DH : DH + 1], 1.0)

            for rep in range(reps):
                # ---- phase A: K projection first (unblocks attention),
                # then V projection ----
                for st, (soff, w) in enumerate(kst):
                    ssl = slice(soff, soff + w)
                    xk_ch = []
                    for dc in range(NDC):
                        dsl = slice(dc * P, (dc + 1) * P)
                        tk = inp.tile([P, QT], BF, tag="xk", name=f"xk_{rep}_{st}_{dc}")
                        nc.sync.dma_start(out=tk[:, :w], in_=xkT[dsl, ssl])
                        xk_ch.append(tk)
                    for ec in range(NEC):
                        esl = slice(ec * P, (ec + 1) * P)
                        psk = psmm.tile(
                            [P, QT], F32, tag="mm", name=f"psk_{rep}_{st}_{ec}"
                        )
                        for dc in range(NDC):
                            nc.tensor.matmul(
                                psk[:, :w], lhsT=wk_sb[:, dc, esl],
                                rhs=xk_ch[dc][:, :w],
                                start=(dc == 0), stop=(dc == NDC - 1),
                            )
                        nc.scalar.activation(
                            out=kt_sb[:, ec, ssl], in_=psk[:, :w],
                            func=Act.Identity, bias=bk_sb[:, ec : ec + 1],
                        )
                for st, (soff, w) in enumerate(kst):
                    ssl = slice(soff, soff + w)
                    xv_ch = []
                    for dc in range(NDC):
                        dsl = slice(dc * P, (dc + 1) * P)
                        tv = inp.tile([P, QT], BF, tag="xv", name=f"xv_{rep}_{st}_{dc}")
                        nc.sync.dma_start(out=tv[:, :w], in_=xvT[dsl, ssl])
                        xv_ch.append(tv)
                    for sc in range(w // P):
                        kc = soff // P + sc
                        psv = psmm.tile([P, EG], F32, tag="mm", name=f"psv_{rep}_{kc}")
                        for dc in range(NDC):
                            nc.tensor.matmul(
                                psv,
                                lhsT=xv_ch[dc][:, sc * P : (sc + 1) * P],
                                rhs=wv_sb[:, dc, :],
                                start=(dc == 0), stop=(dc == NDC - 1),
                            )
                        nc.vector.tensor_tensor(
                            out=va_sb[:, kc, :, 0:DH],
                            in0=psv.rearrange("p (h e) -> p h e", h=NHG),
                            in1=bv_sb.rearrange("p (h e) -> p h e", h=NHG),
                            op=Alu.add,
                        )

                # ---- phase B: Q projection + attention per q-tile ----
                for qt in range(NQT):
                    qsl = slice(qt * QT, (qt + 1) * QT)
                    xq_ch = []
                    for dc in range(NDC):
                        dsl = slice(dc * P, (dc + 1) * P)
                        tq = inp.tile([P, QT], BF, tag="xq", name=f"xq_{rep}_{qt}_{dc}")
                        nc.sync.dma_start(out=tq, in_=xqT[dsl, qsl])
                        xq_ch.append(tq)
                    for ec in range(NEC):
                        esl = slice(ec * P, (ec + 1) * P)
                        psq = psmm.tile(
                            [P, QT], F32, tag="mm", name=f"psq_{rep}_{qt}_{ec}"
                        )
                        for dc in range(NDC):
                            nc.tensor.matmul(
                                psq, lhsT=wq_sb[:, dc, esl], rhs=xq_ch[dc],
                                start=(dc == 0), stop=(dc == NDC - 1),
                            )
                        nc.vector.tensor_scalar_add(
                            qt_sb[:, ec, qsl], psq, bq_sb[:, ec : ec + 1]
                        )
                    for pr in range(NEC):
                        pts = []
                        for kc in range(nkc):
                            ksl = slice(kc * P, (kc + 1) * P)
                            pss = pssc.tile(
                                [P, 2 * QT], F32, tag="sc",
                                name=f"pss_{rep}_{qt}_{pr}_{kc}",
                            )
                            nc.tensor.matmul(
                                pss[:, 0:QT],
                                lhsT=kt_sb[0:64, pr, ksl],
                                rhs=qt_sb[0:64, pr, qsl],
                                start=True, stop=True, tile_position=(0, 0),
                            )
                            nc.tensor.matmul(
                                pss[:, QT : 2 * QT],
                                lhsT=kt_sb[64:128, pr, ksl],
                                rhs=qt_sb[64:128, pr, qsl],
                                start=True, stop=True, tile_position=(64, 0),
                            )
                            pt = ptp.tile(
                                [P, 2 * QT], BF, tag="pt",
                                name=f"pt_{rep}_{qt}_{pr}_{kc}",
                            )
                            nc.scalar.activation(
                                out=pt, in_=pss, func=Act.Exp,
                                bias=mb_sb[:, kc : kc + 1], scale=1.0,
                            )
                            pts.append(pt)

                        for hh in range(2):
                            h = 2 * pr + hh
                            hsl = slice(64 * hh, 64 * (hh + 1))
                            pso = psop.tile(
                                [DH + 1, QT], F32, tag="o", name=f"pso_{rep}_{qt}_{h}"
                            )
                            for kc in range(nkc):
                                nc.tensor.matmul(
                                    pso, lhsT=va_sb[:, kc, h, :],
                                    rhs=pts[kc][:, hh * QT : (hh + 1) * QT],
                                    start=(kc == 0), stop=(kc == nkc - 1),
                                )
                            rec = small.tile(
                                [1, QT], F32, tag="rec", name=f"rec_{rep}_{qt}_{h}"
                            )
                            nc.vector.reciprocal(out=rec, in_=pso[DH : DH + 1, :])
                            rd = dscr.tile(
                                [QT], F32, tag="rd", name=f"rd_{rep}_{qt}_{h}"
                            )
                            nc.sync.dma_start(out=rd, in_=rec)
                            rb = small.tile(
                                [64, QT], F32, tag="rb", name=f"rb_{rep}_{qt}_{h}"
                            )
                            nc.sync.dma_start(
                                out=rb,
                                in_=bass.AP(
                                    tensor=rd.tensor, offset=rd.offset,
                                    ap=[[0, 64]] + list(rd.ap),
                                ),
                            )
                            nc.vector.tensor_tensor(
                                out=ot_sb[hsl, pr, qsl], in0=pso[0:DH, :], in1=rb,
                                op=Alu.mult,
                            )

                # ---- phase C: output projection (partial y) ----
                for j in range(S // P):
                    jsl = slice(j * P, (j + 1) * P)
                    for es in range(2):
                        esl = slice(es * 512, (es + 1) * 512)
                        psy = psmm.tile(
                            [P, 512], F32, tag="mm", name=f"psy_{rep}_{j}_{es}"
                        )
                        for ec in range(NEC):
                            nc.tensor.matmul(
                                psy,
                                lhsT=ot_sb[:, ec, jsl],
                                rhs=wo_sb[:, ec, esl],
                                start=(ec == 0), stop=(ec == NEC - 1),
                            )
                        yv = small.tile(
                            [P, 512], BF, tag="yv", name=f"yv_{rep}_{j}_{es}"
                        )
                        nc.vector.tensor_copy(yv, psy)
                        nc.sync.dma_start(out=y[jsl, esl], in_=yv)

    nc.compile()
    _CACHE[key] = nc
    return nc


def make_in_maps(x_q, x_k, x_v, mask, Wq, bq, Wk, bk, Wv, bv, Wo, bo):
    """Returns (in_maps, sk): per-core input dicts + compacted key length."""
    x_q = np.asarray(x_q, np.float32)
    x_k = np.asarray(x_k, np.float32)
    x_v = np.asarray(x_v, np.float32)
    mask = np.asarray(mask)
    Wq = np.asarray(Wq, np.float32)
    Wk = np.asarray(Wk, np.float32)
    Wv = np.asarray(Wv, np.float32)
    Wo = np.asarray(Wo, np.float32)
    bq = np.asarray(bq, np.float32)
    bk = np.asarray(bk, np.float32)
    bv = np.asarray(bv, np.float32)

    scale = 1.0 / np.sqrt(np.float32(DH))

    idxs = [np.nonzero(mask[b])[0] for b in range(B)]
    nmax = max(1, max(len(ix) for ix in idxs))
    sk = ((nmax + P - 1) // P) * P

    # per-batch tensors are shared by the 4 cores of that batch
    per_batch = []
    for b in range(B):
        ix = idxs[b]
        n = len(ix)
        xkc = np.zeros((D, sk), np.float32)
        xkc[:, :n] = x_k[b].T[:, ix]
        xvc = np.zeros((D, sk), np.float32)
        xvc[:, :n] = x_v[b].T[:, ix]
        mb = np.full(sk, -1e9, np.float32)
        mb[:n] = 0.0
        per_batch.append(
            (x_q[b].T.astype(BF16), xkc.astype(BF16), xvc.astype(BF16), mb)
        )

    in_maps = []
    for c in range(8):
        b, g = c // G, c % G
        sl = slice(g * EG, (g + 1) * EG)
        xqt, xkt, xvt, mb = per_batch[b]
        in_maps.append(
            {
                "xqT": xqt,
                "xkT": xkt,
                "xvT": xvt,
                "wqT": (Wq[sl] * scale).T.astype(BF16),
                "wkT": Wk[sl].T.astype(BF16),
                "wvT": Wv[sl].T.astype(BF16),
                "woT": Wo[:, sl].T.astype(BF16),
                "bqv": (bq[sl] * scale).astype(np.float32),
                "bkv": bk[sl].astype(np.float32),
                "bvv": bv[sl].astype(np.float32),
                "mbv": mb,
            }
        )
    return in_maps, sk


def kernel(x_q, x_k, x_v, mask, Wq, bq, Wk, bk, Wv, bv, Wo, bo):
    from concourse.bass_utils import run_bass_kernel_spmd

    in_maps, sk = make_in_maps(x_q, x_k, x_v, mask, Wq, bq, Wk, bk, Wv, bv, Wo, bo)
    nc = _build(sk)
    bo = np.asarray(bo, np.float32)

    r = run_bass_kernel_spmd(nc, in_maps, core_ids=list(range(8)))
    parts = np.stack(
        [r.results[c]["y"].astype(np.float32) for c in range(8)]
    )  # (8, S, D)
    out = parts.reshape(B, G, S, D).sum(axis=1) + bo
    return out.astype(np.float32)
